# revision 1
# baseline (speedup 1.0000x reference)
"""GATv2 (2-layer, 8-head) message-passing kernel for 8 Trainium2 NeuronCores.

Sharding: nodes (and their incoming edges) are partitioned across the 8 cores
by destination node; weights are replicated; the xl = h @ Wl projection is
computed replicated (cheap) so every core can gather arbitrary source rows
locally via SWDGE dma_gather; segment softmax + scatter-sum are done per-core
with PE indicator-matmuls (edges sorted by destination, grouped into 32-node
blocks padded to a uniform tile count); one AllGather of the updated node
features runs between the two layers. Softmax uses exp without the max shift
(shift-invariant; values are tiny). lrelu(x) = 0.6x + 0.4|x| with the 0.6
folded into the attention constant and the |.| scale.

bf16 is used for the edge phase (gather payload, feature matmul operands,
elementwise chain); all accumulations (PSUM, reductions, residual h) are fp32.

Self-contained: takes full (unsharded) inputs, returns the full output.
"""

import os
import numpy as np
from contextlib import ExitStack

import ml_dtypes
import concourse.bass as bass
import concourse.tile as tile
from concourse import bacc, mybir
from concourse.bass_utils import run_bass_kernel_spmd

# Problem constants (from the nn_GATv2Model spec)
N = 10000
E = 320000
F_IN = 7
D = 256
H = 8
C = 32
L = 2
G = 16
SLOPE = 0.2

NCORES = 8
NPC_REAL = 1250          # real nodes per core
NPC = 1280               # padded nodes per core (multiple of 128)
NP = NCORES * NPC        # padded total nodes
BLK = 32                 # dst nodes per block (indicator-matmul segment group)
NBLK = NPC // BLK        # blocks per core
PT = 128                 # edges per tile (partition dim)

F32 = mybir.dt.float32
BF16 = mybir.dt.bfloat16
I16 = mybir.dt.int16
NBF = ml_dtypes.bfloat16


def _pad_map():
    n = np.arange(N)
    return (n // NPC_REAL) * NPC + (n % NPC_REAL)


def _prep_edges(edge_index: np.ndarray):
    """Sort/shard/pad edges. Returns (T_blk, per-core dict lists)."""
    pm = _pad_map()
    src_p = pm[edge_index[0]]
    dst_p = pm[edge_index[1]]
    owner = dst_p // NPC

    per_core = []
    t_blk = 1
    for c in range(NCORES):
        sel = owner == c
        es = src_p[sel]
        ed = dst_p[sel] - c * NPC
        order = np.argsort(ed, kind="stable")
        es, ed = es[order], ed[order]
        blk = ed // BLK
        cnt = np.bincount(blk, minlength=NBLK)
        t_blk = max(t_blk, int(np.ceil(cnt.max() / PT)))
        per_core.append((es, ed, blk, cnt))

    cores = []
    ne_slots = NBLK * t_blk * PT
    for c in range(NCORES):
        es, ed, blk, cnt = per_core[c]
        starts = np.zeros(NBLK, dtype=np.int64)
        starts[1:] = np.cumsum(cnt)[:-1]
        within = np.arange(len(es)) - starts[blk]
        slot = blk * (t_blk * PT) + within

        src_slots = np.zeros(ne_slots, dtype=np.int64)
        dl32 = np.full(ne_slots, -1, dtype=np.int64)
        src_slots[slot] = es
        dl32[slot] = ed - blk * BLK

        # wrapped gather indices [128, ne_slots//16]: within each block,
        # idx j at [j%16, j//16]
        w16 = np.concatenate(
            [src_slots[b * t_blk * PT:(b + 1) * t_blk * PT]
             .reshape(t_blk * 8, 16).T
             for b in range(NBLK)], axis=1).astype(np.int16)
        srcw = np.tile(w16, (8, 1))

        # indicator [NT, 128, 32] -> partition-major [128, NT, 32] bf16
        nt = NBLK * t_blk
        ind = np.zeros((ne_slots, BLK), dtype=np.float32)
        valid = dl32 >= 0
        ind[np.nonzero(valid)[0], dl32[valid]] = 1.0
        ind = ind.reshape(nt, PT, BLK)
        ind_pm = np.ascontiguousarray(ind.transpose(1, 0, 2)).astype(NBF)
        # transposed indicator [32, NT, 128] bf16 (base partition 0)
        indt_pm = np.ascontiguousarray(
            ind.transpose(0, 2, 1).transpose(1, 0, 2)).astype(NBF)
        cores.append({"srcw": srcw, "ind_pm": ind_pm, "indt_pm": indt_pm})
    return t_blk, cores


def _build(t_blk: int, debug: bool = False):
    """Build + compile the SPMD Bass program (identical on all cores)."""
    STAGE = int(os.environ.get("GAT_STAGE", "99"))
    nt = NBLK * t_blk
    ne_slots = nt * PT
    t8 = t_blk * 8
    QW = 4                      # tiles per quad-group for DVE/ACT ops

    nc = bacc.Bacc("TRN2", target_bir_lowering=False, debug=False,
                   num_devices=NCORES)

    # ---- external inputs ----
    xT = nc.dram_tensor("xT", [F_IN, NP], F32, kind="ExternalInput").ap()
    xTown = nc.dram_tensor("xTown", [F_IN, NPC], F32, kind="ExternalInput").ap()
    wp = nc.dram_tensor("wp", [F_IN, D], F32, kind="ExternalInput").ap()
    wl = nc.dram_tensor("wl", [128, L, 2, D + 8], BF16, kind="ExternalInput").ap()
    wr = nc.dram_tensor("wr", [128, L, 2, D], F32, kind="ExternalInput").ap()
    attrep = nc.dram_tensor("attrep", [128, L, 4, D], BF16, kind="ExternalInput").ap()
    bcols = nc.dram_tensor("bcols", [128, 2 + 2 * L + 2], F32, kind="ExternalInput").ap()
    i128 = nc.dram_tensor("i128", [128, 128], BF16, kind="ExternalInput").ap()
    i32 = nc.dram_tensor("i32", [32, 32], F32, kind="ExternalInput").ap()
    epsc = nc.dram_tensor("epsc", [32, 1], F32, kind="ExternalInput").ap()
    srcw_d = nc.dram_tensor("srcw", [128, ne_slots // 16], I16, kind="ExternalInput").ap()
    ind_d = nc.dram_tensor("ind", [128, nt, BLK], BF16, kind="ExternalInput").ap()
    indt_d = nc.dram_tensor("indt", [32, nt, PT], BF16, kind="ExternalInput").ap()

    # ---- internal DRAM ----
    xl_d = nc.dram_tensor("xl_d", [NP, 384], BF16).ap()
    xr_d = nc.dram_tensor("xr_d", [NPC, D], BF16).ap()
    h_cur = nc.dram_tensor("h_cur", [NCORES, 2, 128, NPC], BF16).ap()
    h_upd = nc.dram_tensor("h_upd", [2, 128, NPC], BF16).ap()
    h_all = nc.dram_tensor("h_all", [NCORES, 2, 128, NPC], BF16,
                           addr_space="Shared").ap()

    # ---- outputs ----
    y_out = nc.dram_tensor("y_out", [1, NPC], F32, kind="ExternalOutput").ap()
    if debug:
        hdump = nc.dram_tensor("hdump", [L, 2, 128, NPC], F32, kind="ExternalOutput").ap()

    with tile.TileContext(nc) as tc, ExitStack() as ctx:
        cpool = ctx.enter_context(tc.tile_pool(name="consts", bufs=1))
        stream = ctx.enter_context(tc.tile_pool(name="stream", bufs=6))
        drain = ctx.enter_context(tc.tile_pool(name="drain", bufs=4))
        gath = ctx.enter_context(tc.tile_pool(name="gath", bufs=3))
        indp = ctx.enter_context(tc.tile_pool(name="indp", bufs=3))
        indtp = ctx.enter_context(tc.tile_pool(name="indtp", bufs=3))
        xrp = ctx.enter_context(tc.tile_pool(name="xrp", bufs=3))
        upool = ctx.enter_context(tc.tile_pool(name="upool", bufs=3))
        lfpool = ctx.enter_context(tc.tile_pool(name="lfpool", bufs=3))
        appool = ctx.enter_context(tc.tile_pool(name="appool", bufs=3))
        epool = ctx.enter_context(tc.tile_pool(name="epool", bufs=3))
        astp = ctx.enter_context(tc.tile_pool(name="astp", bufs=3))
        smallp = ctx.enter_context(tc.tile_pool(name="smallp", bufs=3))
        outbp = ctx.enter_context(tc.tile_pool(name="outbp", bufs=3))

        # PSUM budget (8 banks): pfeat quads 2x2=4 (also reused as the
        # matmul-phase accumulators via the same tag), pblk 3, ptr 1
        pfeat = ctx.enter_context(tc.tile_pool(name="pfeat", bufs=2, space="PSUM"))
        pblk = ctx.enter_context(tc.tile_pool(name="pblk", bufs=3, space="PSUM"))
        ptrp = ctx.enter_context(tc.tile_pool(name="ptrp", bufs=1, space="PSUM"))

        def pmm_tile():
            t = pfeat.tile([128, 2, 512], F32, tag="pf")
            return t[:, 0, :]

        # ---- resident SBUF constants ----
        wp_sb = cpool.tile([F_IN, D], F32)
        nc.sync.dma_start(wp_sb[:], wp)
        wl_sb = cpool.tile([128, L, 2, D + 8], BF16)
        nc.sync.dma_start(wl_sb[:], wl)
        wr_sb = cpool.tile([128, L, 2, D], F32)
        nc.sync.dma_start(wr_sb[:], wr)
        att_sb = cpool.tile([128, L, 4, D], BF16)
        nc.sync.dma_start(att_sb[:], attrep)
        bc_sb = cpool.tile([128, 2 + 2 * L + 2], F32)
        nc.sync.dma_start(bc_sb[:], bcols)
        i128_sb = cpool.tile([128, 128], BF16)
        nc.sync.dma_start(i128_sb[:], i128)
        i32_sb = cpool.tile([32, 32], F32)
        nc.sync.dma_start(i32_sb[:], i32)
        eps_sb = cpool.tile([32, 1], F32)
        nc.sync.dma_start(eps_sb[:], epsc)
        srcw_sb = cpool.tile([128, ne_slots // 16], I16)
        nc.sync.dma_start(srcw_sb[:], srcw_d)

        h_own = cpool.tile([128, 2, NPC], F32)     # h_T own slice, resident

        def drain_ps(dst_ap, ps_ap, parity, bias=None):
            """PSUM -> SBUF copy alternating ACT/DVE to balance load."""
            if bias is not None:
                nc.scalar.activation(dst_ap, ps_ap,
                                     mybir.ActivationFunctionType.Identity,
                                     bias=bias)
            elif parity % 2 == 0:
                nc.scalar.activation(dst_ap, ps_ap,
                                     mybir.ActivationFunctionType.Copy)
            else:
                nc.vector.tensor_copy(dst_ap, ps_ap)

        # ---- P0: h0 = x @ Wp + bp (full, replicated) -> h_cur (bf16) ----
        di = 0
        for b in range(NCORES):
            for k in range(4):
                xs = stream.tile([F_IN, 320], F32, tag="xs")
                nc.sync.dma_start(xs[:], xT[:, b * NPC + 320 * k:b * NPC + 320 * (k + 1)])
                for ch in range(2):
                    ps = pmm_tile()
                    nc.tensor.matmul(ps[:, 0:320], wp_sb[:, 128 * ch:128 * (ch + 1)],
                                     xs[:], start=True, stop=True)
                    hsb = drain.tile([128, 320], BF16, tag="hsb")
                    drain_ps(hsb[:], ps[:, 0:320], di, bias=bc_sb[:, ch:ch + 1])
                    di += 1
                    nc.sync.dma_start(h_cur[b, ch, :, 320 * k:320 * (k + 1)], hsb[:])
        # P0b: h0 own slice -> resident SBUF (fp32)
        for k in range(4):
            xs = stream.tile([F_IN, 320], F32, tag="xs")
            nc.sync.dma_start(xs[:], xTown[:, 320 * k:320 * (k + 1)])
            for ch in range(2):
                ps = pmm_tile()
                nc.tensor.matmul(ps[:, 0:320], wp_sb[:, 128 * ch:128 * (ch + 1)],
                                 xs[:], start=True, stop=True)
                drain_ps(h_own[:, ch, 320 * k:320 * (k + 1)], ps[:, 0:320], di,
                         bias=bc_sb[:, ch:ch + 1])
                di += 1

        # ---- layers ----
        for l in range(L if STAGE >= 2 else 0):
            hsrc = h_cur if l == 0 else h_all

            # xl_aug = h @ [Wl|0.6*Wl@A]  (node-major, bf16) -> xl_d cols 0:264
            for jp in range(40):
                ps2 = pfeat.tile([128, 2, 512], F32, tag="pf")
                for o in range(2):
                    j = 2 * jp + o
                    bb, lc = j // 10, (j % 10) * 128
                    for ch in range(2):
                        ht = stream.tile([128, 128], BF16, tag="ht")
                        nc.sync.dma_start(ht[:], hsrc[bb, ch, :, lc:lc + 128])
                        nc.tensor.matmul(ps2[:, o, 0:D + 8], ht[:],
                                         wl_sb[:, l, ch, :],
                                         start=(ch == 0), stop=(ch == 1))
                xsb = drain.tile([128, 2, D + 8], BF16, tag="xsb")
                drain_ps(xsb[:], ps2[:, :, 0:D + 8], jp)
                row0 = 2 * jp * 128
                nc.sync.dma_start(
                    xl_d[row0:row0 + 256, 0:D + 8].rearrange(
                        "(a p) d -> p a d", p=128),
                    xsb[:])

            # xr = h_own @ Wr -> xr_d (node-major bf16, via DRAM round-trip)
            for jo in range(10 if STAGE >= 3 else 0):
                ps = pmm_tile()
                for ch in range(2):
                    nc.tensor.matmul(ps[:, 0:D], h_own[:, ch, 128 * jo:128 * (jo + 1)],
                                     wr_sb[:, l, ch, :],
                                     start=(ch == 0), stop=(ch == 1))
                xrsb = drain.tile([128, D], BF16, tag="xrsb")
                drain_ps(xrsb[:], ps[:, 0:D], jo)
                nc.sync.dma_start(xr_d[128 * jo:128 * (jo + 1), :], xrsb[:])

            # edge sweep
            nblk_run = NBLK if STAGE >= 4 else 0
            nblk_run = int(os.environ.get("GAT_NBLK", nblk_run)) if STAGE == 4 else nblk_run
            for b in range(nblk_run):
                xsrc = gath.tile([128, t_blk, 384], BF16)
                for g0 in range(0, t_blk, 8):
                    gn = min(8, t_blk - g0)
                    nc.gpsimd.dma_gather(
                        out_ap=xsrc[:, g0:g0 + gn, :],
                        in_ap=xl_d,
                        idxs_ap=srcw_sb[:, b * t8 + g0 * 8:b * t8 + (g0 + gn) * 8],
                        num_idxs=gn * PT,
                        num_idxs_reg=gn * PT,
                        elem_size=384,
                    )
                ind_sb = indp.tile([128, t_blk, BLK], BF16)
                nc.sync.dma_start(ind_sb[:], ind_d[:, b * t_blk:(b + 1) * t_blk, :])
                indt_sb = indtp.tile([32, t_blk, PT], BF16)
                nc.sync.dma_start(
                    indt_sb[:], indt_d[:, b * t_blk:(b + 1) * t_blk, :])
                xrb_t = xrp.tile([32, D], BF16)
                nc.sync.dma_start(xrb_t[:], xr_d[BLK * b:BLK * (b + 1), :])

                ast = astp.tile([128, t_blk, 8 + D], BF16)
                pb = pblk.tile([32, 8 + D], F32)

                for q0 in range(0, t_blk, QW):
                    w = min(QW, t_blk - q0)
                    pf0 = pfeat.tile([128, 2, 512], F32, tag="pf")
                    pf = pf0[:].rearrange("p a (b d) -> p (a b) d", d=D)
                    for tt in range(q0, q0 + w):
                        nc.tensor.matmul(pf[:, tt - q0, :], indt_sb[:, tt, :],
                                         xrb_t[:], start=True, stop=False)
                        nc.tensor.matmul(pf[:, tt - q0, :], i128_sb[:],
                                         xsrc[:, tt, 0:D],
                                         start=False, stop=True)
                    # u = |feat|; R = sum_c (0.4*att)*u; e = R + SL[src]
                    # (the dst-side linear term cancels in the softmax)
                    u = upool.tile([128, QW, D], BF16)
                    nc.scalar.activation(u[:, 0:w, :], pf[:, 0:w, :],
                                         mybir.ActivationFunctionType.Abs)
                    ap_t = appool.tile([128, QW, D], BF16)
                    nc.vector.tensor_mul(ap_t[:, 0:w, :], u[:, 0:w, :],
                                         att_sb[:, l, 0:w, :])
                    e_sb = epool.tile([128, QW, 8], F32)
                    nc.vector.tensor_reduce(
                        e_sb[:, 0:w, :],
                        ap_t[:, 0:w, :].rearrange("p a (h c) -> p a h c", h=H),
                        axis=mybir.AxisListType.X, op=mybir.AluOpType.add)
                    nc.vector.tensor_add(e_sb[:, 0:w, :], e_sb[:, 0:w, :],
                                         xsrc[:, q0:q0 + w, D:D + 8])
                    # a = exp(e) -> astore[:, t, 0:8] (bf16)
                    nc.scalar.activation(ast[:, q0:q0 + w, 0:8], e_sb[:, 0:w, :],
                                         mybir.ActivationFunctionType.Exp)
                    # msg = a (x) xsrc -> astore[:, t, 8:264] (bf16)
                    nc.vector.tensor_mul(
                        ast[:, q0:q0 + w, 8:].rearrange("p a (h c) -> p a h c", h=H),
                        xsrc[:, q0:q0 + w, 0:D].rearrange("p a (h c) -> p a h c", h=H),
                        ast[:, q0:q0 + w, 0:8].unsqueeze(-1).broadcast_to(
                            [128, w, H, C]))
                    for tt in range(q0, q0 + w):
                        nc.tensor.matmul(pb[:], ind_sb[:, tt, :], ast[:, tt, :],
                                         start=(tt == 0), stop=(tt == t_blk - 1))

                # block epilogue: denom -> 1/(denom+eps); scale; transpose; h +=
                dsb = smallp.tile([32, 8], F32, tag="dsb")
                nc.scalar.activation(dsb[:], pb[:, 0:8],
                                     mybir.ActivationFunctionType.Identity,
                                     bias=eps_sb[:])
                dinv = smallp.tile([32, 8], F32, tag="dinv")
                nc.vector.reciprocal(dinv[:], dsb[:])
                outb = outbp.tile([32, D], F32)
                nc.vector.tensor_mul(
                    outb[:].rearrange("p (h c) -> p h c", h=H),
                    pb[:, 8:].rearrange("p (h c) -> p h c", h=H),
                    dinv[:].unsqueeze(-1).broadcast_to([32, H, C]))
                pt = ptrp.tile([128, 2, 32], F32)
                nc.tensor.transpose(pt[:, 0, :], outb[:, 0:128], i32_sb[:])
                nc.tensor.transpose(pt[:, 1, :], outb[:, 128:256], i32_sb[:])
                for ch in range(2):
                    nc.vector.tensor_add(h_own[:, ch, 32 * b:32 * (b + 1)],
                                         h_own[:, ch, 32 * b:32 * (b + 1)],
                                         pt[:, ch, :])

            # bconv
            for ch in range(2):
                nc.vector.tensor_scalar_add(h_own[:, ch, :], h_own[:, ch, :],
                                            bc_sb[:, 2 + 2 * l + ch:3 + 2 * l + ch])

            if debug:
                nc.sync.dma_start(hdump[l].rearrange("c p n -> p c n"), h_own[:])

            if l == 0 and STAGE >= 5:
                # cast h_own -> bf16 and AllGather for layer 2
                hupd_sb = drain.tile([128, 2, NPC], BF16, tag="hupd")
                nc.scalar.activation(hupd_sb[:, 0, :], h_own[:, 0, :],
                                     mybir.ActivationFunctionType.Copy)
                nc.vector.tensor_copy(hupd_sb[:, 1, :], h_own[:, 1, :])
                nc.sync.dma_start(h_upd.rearrange("c p n -> p c n"), hupd_sb[:])
                nc.gpsimd.collective_compute(
                    "AllGather", mybir.AluOpType.bypass,
                    replica_groups=[list(range(NCORES))],
                    ins=[h_upd], outs=[h_all])

        # ---- final: y = h_own @ Wpred ----
        for k in range(3):
            w = 512 if k < 2 else NPC - 1024
            ps = pmm_tile()
            for ch in range(2):
                nc.tensor.matmul(ps[0:1, 0:w], bc_sb[:, 6 + ch:7 + ch],
                                 h_own[:, ch, 512 * k:512 * k + w],
                                 start=(ch == 0), stop=(ch == 1))
            ysb = drain.tile([1, 512], F32, tag="ysb")
            nc.scalar.activation(ysb[0:1, 0:w], ps[0:1, 0:w],
                                 mybir.ActivationFunctionType.Copy)
            nc.sync.dma_start(y_out[0:1, 512 * k:512 * k + w], ysb[0:1, 0:w])

    nc.compile()
    return nc


def _host_inputs(x, Wp, bp, Wl, Wr, att, bconv, Wpred):
    pm = _pad_map()
    xp = np.zeros((NP, F_IN), dtype=np.float32)
    xp[pm] = np.asarray(x, dtype=np.float32)
    xT = np.ascontiguousarray(xp.T)

    # augmented projections: [W | 0.6 * W @ blockdiag(att)] so the linear
    # 0.6*sum(att*feat) term rides along with the gather/expand (8 extra cols)
    wl_p = np.zeros((128, L, 2, D + 8), dtype=np.float32)
    wr_p = np.zeros((128, L, 2, D), dtype=np.float32)
    att_p = np.zeros((128, L, 4, D), dtype=np.float32)
    for l in range(L):
        A = np.zeros((D, H), dtype=np.float64)
        for h in range(H):
            A[h * C:(h + 1) * C, h] = 0.6 * att[l][h]
        wla = np.concatenate([Wl[l], Wl[l] @ A], axis=1)  # [256, 264]
        for ch in range(2):
            wl_p[:, l, ch, :] = wla[128 * ch:128 * (ch + 1), :]
            wr_p[:, l, ch, :] = Wr[l][128 * ch:128 * (ch + 1), :]
        a = (0.4 * att[l]).reshape(H * C)   # 0.4 factor of lrelu folded here
        for q in range(4):
            att_p[:, l, q, :] = a[None, :]

    bcols = np.zeros((128, 2 + 2 * L + 2), dtype=np.float32)
    for ch in range(2):
        bcols[:, ch] = bp[128 * ch:128 * (ch + 1)]
        for l in range(L):
            bcols[:, 2 + 2 * l + ch] = bconv[l][128 * ch:128 * (ch + 1)]
        bcols[:, 6 + ch] = Wpred[128 * ch:128 * (ch + 1), 0]

    shared = {
        "xT": xT,
        "wp": np.asarray(Wp, dtype=np.float32),
        "wl": wl_p.astype(NBF), "wr": wr_p, "attrep": att_p.astype(NBF),
        "bcols": bcols,
        "i128": np.eye(128, dtype=np.float32).astype(NBF),
        "i32": np.eye(32, dtype=np.float32),
        "epsc": np.full((32, 1), 1e-16, dtype=np.float32),
    }
    xTowns = [np.ascontiguousarray(xT[:, c * NPC:(c + 1) * NPC]) for c in range(NCORES)]
    return shared, xTowns


_CACHE = {}


def kernel(x, edge_index, batch, Wp, bp, Wl, Wr, att, bconv, Wpred, bpred,
           debug=False, _timing=None):
    x = np.asarray(x)
    edge_index = np.asarray(edge_index).astype(np.int64)
    batch = np.asarray(batch).astype(np.int64)

    t_blk, cores = _prep_edges(edge_index)
    shared, xTowns = _host_inputs(np.asarray(x), np.asarray(Wp), np.asarray(bp),
                                  np.asarray(Wl), np.asarray(Wr), np.asarray(att),
                                  np.asarray(bconv), np.asarray(Wpred))

    key = (t_blk, bool(debug), os.environ.get("GAT_STAGE", "99"),
           os.environ.get("GAT_NBLK", ""))
    if key not in _CACHE:
        _CACHE[key] = _build(t_blk, debug=debug)
    nc = _CACHE[key]

    in_maps = []
    for c in range(NCORES):
        m = dict(shared)
        m["xTown"] = xTowns[c]
        m["srcw"] = cores[c]["srcw"]
        m["ind"] = cores[c]["ind_pm"]
        m["indt"] = cores[c]["indt_pm"]
        in_maps.append(m)

    kw = {k: v for k, v in (_timing or {}).items() if k != "result"}
    res = run_bass_kernel_spmd(nc, in_maps, list(range(NCORES)), **kw)
    if _timing is not None:
        _timing["result"] = res

    ycat = np.concatenate([res.results[c]["y_out"][0] for c in range(NCORES)])
    pm = _pad_map()
    y_real = ycat[pm]
    sums = np.bincount(batch, weights=y_real.astype(np.float64), minlength=G)
    cnt = np.bincount(batch, minlength=G).astype(np.float64)
    out = sums / np.maximum(cnt, 1.0) + float(np.asarray(bpred).reshape(-1)[0])
    if debug:
        return out.astype(np.float32)[:, None], res
    return out.astype(np.float32)[:, None]



# revision 7
# speedup vs baseline: 1.3039x; 1.3039x over previous
"""GATv2 (2-layer, 8-head) message-passing kernel for 8 Trainium2 NeuronCores.

Sharding: nodes (and incoming edges) are partitioned across 8 cores by
destination; weights replicated.  Per core, dst nodes are LPT-bin-packed into
40 blocks of 32 nodes (balancing edge counts so every block needs the same
t_blk=8 tiles of 128 edges).  Layer-0 source features xl0 are computed
replicated directly from x via the host-folded weight [Wp;bp]@[Wl|Wl@A]
(contraction dim 8), written to DRAM, and gathered per edge with SWDGE.
Layer-1 xl is NOT recomputed replicated: each core projects its own 1280
updated nodes and a chunked AllGather (5 chunks of 8 blocks, issued as the
edge-0 sweep retires blocks) distributes them, overlapping the collective
with compute.

Edge math per 128-edge tile: ACT preloads the gathered xl into PSUM, one PE
matmul accumulates the indicator-expanded xr on top (feat = xl[src]+xr[dst]);
ACT abs, DVE mul by 0.4*att + per-head reduce (lrelu folded as 0.6x+0.4|x|
with the 0.6-linear source term riding in 8 extra gathered columns; the dst
linear term cancels in the softmax), ACT exp, DVE alpha*xl, and a PE
indicator-matmul scatters [a | msg] into per-block PSUM accumulators.

bf16 everywhere on the edge path; accumulations (PSUM, h residual) fp32.
Self-contained: takes full (unsharded) inputs, returns the full output.
"""

import os
import numpy as np
from contextlib import ExitStack

import ml_dtypes
import concourse.bass as bass
import concourse.tile as tile
from concourse import bacc, mybir
from concourse.bass_utils import run_bass_kernel_spmd

# Problem constants (from the nn_GATv2Model spec)
N = 10000
E = 320000
F_IN = 7
D = 256
H = 8
C = 32
L = 2
G = 16
SLOPE = 0.2

NCORES = 8
NPC_REAL = 1250          # real nodes per core
NPC = 1280               # padded nodes per core (multiple of 256)
NP = NCORES * NPC        # padded total nodes
BLK = 32                 # dst nodes per block (indicator-matmul segment group)
NBLK = NPC // BLK        # blocks per core
PT = 128                 # edges per tile (partition dim)
CH = 5                   # AllGather chunks (NPC/256)
CHN = 256                # nodes per chunk
QW = 4                   # tiles per PSUM quad-group
DA = D + 8               # payload cols: 256 feat + 8 source-linear
DPAD = 384               # xl_d row pitch in bf16 elems (768B, 256B multiple)

F32 = mybir.dt.float32
BF16 = mybir.dt.bfloat16
I16 = mybir.dt.int16
NBF = ml_dtypes.bfloat16


def _pad_map():
    n = np.arange(N)
    return (n // NPC_REAL) * NPC + (n % NPC_REAL)


def _prep_edges(edge_index: np.ndarray):
    """LPT-pack dst nodes into blocks, sort/shard/pad edges.

    Returns (t_blk, perms, cores) where perms[c] is old_of_new (new local
    id -> old padded-local id) and cores[c] has srcw/ind_pm/indt_pm."""
    pm = _pad_map()
    src_p = pm[edge_index[0]]
    dst_p = pm[edge_index[1]]
    owner = dst_p // NPC

    # ---- per-core LPT bin-packing of dst nodes into NBLK bins of 32 ----
    perms_new = []      # new_of_old per core
    perms_old = []      # old_of_new per core
    core_sel = []
    t_blk = 1
    for c in range(NCORES):
        sel = owner == c
        core_sel.append(sel)
        dl = dst_p[sel] - c * NPC
        deg = np.bincount(dl, minlength=NPC)
        order = np.argsort(-deg, kind="stable")
        binload = np.zeros(NBLK, np.int64)
        bincnt = np.zeros(NBLK, np.int64)
        assign = np.zeros(NPC, np.int64)
        for node in order:
            open_b = np.nonzero(bincnt < BLK)[0]
            b = open_b[np.argmin(binload[open_b])]
            assign[node] = b
            binload[b] += deg[node]
            bincnt[b] += 1
        new_of_old = np.zeros(NPC, np.int64)
        pos = np.zeros(NBLK, np.int64)
        for old in range(NPC):
            b = assign[old]
            new_of_old[old] = b * BLK + pos[b]
            pos[b] += 1
        perms_new.append(new_of_old)
        perms_old.append(np.argsort(new_of_old))
        t_blk = max(t_blk, int(np.ceil(binload.max() / PT)))

    # global chunk-major row map: padded id -> xl_d row
    row_of = np.zeros(NP, np.int64)
    for c in range(NCORES):
        r = perms_new[c]                      # old local -> new local
        j = r // CHN
        row_of[c * NPC:(c + 1) * NPC] = (j * NCORES + c) * CHN + (r % CHN)

    t8 = t_blk * 8
    ne_slots = NBLK * t_blk * PT
    cores = []
    for c in range(NCORES):
        sel = core_sel[c]
        es = row_of[src_p[sel]]                       # global xl_d rows
        ed = perms_new[c][dst_p[sel] - c * NPC]       # new local dst
        order = np.argsort(ed, kind="stable")
        es, ed = es[order], ed[order]
        blk = ed // BLK
        cnt = np.bincount(blk, minlength=NBLK)
        starts = np.zeros(NBLK, dtype=np.int64)
        starts[1:] = np.cumsum(cnt)[:-1]
        within = np.arange(len(es)) - starts[blk]
        slot = blk * (t_blk * PT) + within

        src_slots = np.zeros(ne_slots, dtype=np.int64)
        dl32 = np.full(ne_slots, -1, dtype=np.int64)
        src_slots[slot] = es
        dl32[slot] = ed - blk * BLK

        # wrapped gather indices [128, ne_slots//16]
        w16 = np.concatenate(
            [src_slots[b * t_blk * PT:(b + 1) * t_blk * PT]
             .reshape(t_blk * 8, 16).T
             for b in range(NBLK)], axis=1).astype(np.int16)
        srcw = np.tile(w16, (8, 1))

        # indicator [NT, 128, 32] -> partition-major [128, NT, 32] bf16
        nt = NBLK * t_blk
        ind = np.zeros((ne_slots, BLK), dtype=np.float32)
        valid = dl32 >= 0
        ind[np.nonzero(valid)[0], dl32[valid]] = 1.0
        ind = ind.reshape(nt, PT, BLK)
        ind_pm = np.ascontiguousarray(ind.transpose(1, 0, 2)).astype(NBF)
        indt_pm = np.ascontiguousarray(
            ind.transpose(0, 2, 1).transpose(1, 0, 2)).astype(NBF)
        cores.append({"srcw": srcw, "ind_pm": ind_pm, "indt_pm": indt_pm})
    return t_blk, perms_old, perms_new, row_of, cores


def _build(t_blk: int, debug: bool = False):
    """Build + compile the SPMD Bass program (identical on all cores)."""
    STAGE = int(os.environ.get("GAT_STAGE", "99"))
    PRELOAD = os.environ.get("GAT_PRELOAD", "1") == "1"
    EBF16 = os.environ.get("GAT_EBF16", "1") == "1"
    nt = NBLK * t_blk
    t8 = t_blk * 8

    nc = bacc.Bacc("TRN2", target_bir_lowering=False, debug=False,
                   num_devices=NCORES)

    # ---- external inputs ----
    xaug = nc.dram_tensor("xaug", [F_IN + 1, NP], BF16, kind="ExternalInput").ap()
    xtown = nc.dram_tensor("xtown", [F_IN + 1, NPC], F32, kind="ExternalInput").ap()
    wp = nc.dram_tensor("wp", [F_IN + 1, D], F32, kind="ExternalInput").ap()
    wf0 = nc.dram_tensor("wf0", [F_IN + 1, DA], BF16, kind="ExternalInput").ap()
    wl1 = nc.dram_tensor("wl1", [128, 2, DA], F32, kind="ExternalInput").ap()
    wr = nc.dram_tensor("wr", [128, L, 2, D], F32, kind="ExternalInput").ap()
    attrep = nc.dram_tensor("attrep", [128, L, QW, D], BF16, kind="ExternalInput").ap()
    bcols = nc.dram_tensor("bcols", [128, 8], F32, kind="ExternalInput").ap()
    i32 = nc.dram_tensor("i32", [32, 32], F32, kind="ExternalInput").ap()
    epsc = nc.dram_tensor("epsc", [32, 1], F32, kind="ExternalInput").ap()
    i128 = nc.dram_tensor("i128", [128, 128], BF16, kind="ExternalInput").ap()
    srcw_d = nc.dram_tensor("srcw", [128, NBLK * t8], I16, kind="ExternalInput").ap()
    ind_d = nc.dram_tensor("ind", [128, nt, BLK], BF16, kind="ExternalInput").ap()
    indt_d = nc.dram_tensor("indt", [32, nt, PT], BF16, kind="ExternalInput").ap()

    # ---- internal DRAM ----
    xl_d0 = nc.dram_tensor("xl_d0", [NP, DPAD], BF16).ap()
    stage = nc.dram_tensor("stage", [NPC, DPAD], BF16).ap()
    xl_d1 = nc.dram_tensor("xl_d1", [NP, DPAD], BF16, addr_space="Shared").ap()

    # ---- outputs ----
    y_out = nc.dram_tensor("y_out", [1, NPC], F32, kind="ExternalOutput").ap()
    if debug:
        hdump = nc.dram_tensor("hdump", [L, 2, 128, NPC], F32, kind="ExternalOutput").ap()

    with tile.TileContext(nc) as tc, ExitStack() as ctx:
        cpool = ctx.enter_context(tc.tile_pool(name="consts", bufs=1))
        drain = ctx.enter_context(tc.tile_pool(name="drain", bufs=4))
        gath = ctx.enter_context(tc.tile_pool(name="gath", bufs=3))
        indp = ctx.enter_context(tc.tile_pool(name="indp", bufs=3))
        indtp = ctx.enter_context(tc.tile_pool(name="indtp", bufs=3))
        xrpool = ctx.enter_context(tc.tile_pool(name="xrpool", bufs=1))
        upool = ctx.enter_context(tc.tile_pool(name="upool", bufs=3))
        appool = ctx.enter_context(tc.tile_pool(name="appool", bufs=3))
        epool = ctx.enter_context(tc.tile_pool(name="epool", bufs=3))
        astp = ctx.enter_context(tc.tile_pool(name="astp", bufs=3))
        smallp = ctx.enter_context(tc.tile_pool(name="smallp", bufs=3))
        outbp = ctx.enter_context(tc.tile_pool(name="outbp", bufs=3))

        # PSUM (8 banks): pfeat 2x2=4, pblk 3, ptr 1
        pfeat = ctx.enter_context(tc.tile_pool(name="pfeat", bufs=2, space="PSUM"))
        pblk = ctx.enter_context(tc.tile_pool(name="pblk", bufs=3, space="PSUM"))
        ptrp = ctx.enter_context(tc.tile_pool(name="ptrp", bufs=1, space="PSUM"))

        def pmm_tile():
            t = pfeat.tile([128, QW, D], F32, tag="pf")
            return t[:].rearrange("p a b -> p (a b)")

        # ---- resident SBUF constants ----
        x_sb = cpool.tile([F_IN + 1, NP], BF16)
        nc.sync.dma_start(x_sb[:], xaug)
        xto_sb = cpool.tile([F_IN + 1, NPC], F32)
        nc.sync.dma_start(xto_sb[:], xtown)
        wp_sb = cpool.tile([F_IN + 1, D], F32)
        nc.sync.dma_start(wp_sb[:], wp)
        wf0_sb = cpool.tile([F_IN + 1, DA], BF16)
        nc.sync.dma_start(wf0_sb[:], wf0)
        wl1_sb = cpool.tile([128, 2, DA], F32)
        nc.sync.dma_start(wl1_sb[:], wl1)
        wr_sb = cpool.tile([128, L, 2, D], F32)
        nc.sync.dma_start(wr_sb[:], wr)
        att_sb = cpool.tile([128, L, QW, D], BF16)
        nc.sync.dma_start(att_sb[:], attrep)
        bc_sb = cpool.tile([128, 8], F32)
        nc.sync.dma_start(bc_sb[:], bcols)
        i32_sb = cpool.tile([32, 32], F32)
        nc.sync.dma_start(i32_sb[:], i32)
        eps_sb = cpool.tile([32, 1], F32)
        nc.sync.dma_start(eps_sb[:], epsc)
        i128_sb = cpool.tile([128, 128], BF16)
        nc.sync.dma_start(i128_sb[:], i128)
        srcw_sb = cpool.tile([128, NBLK * t8], I16)
        nc.sync.dma_start(srcw_sb[:], srcw_d)

        h_own = cpool.tile([128, 2, NPC], F32)     # h own slice, resident

        di = [0]

        def drain_ps(dst_ap, ps_ap):
            """PSUM -> SBUF copy alternating ACT/DVE to balance load."""
            if di[0] % 2 == 0:
                nc.scalar.activation(dst_ap, ps_ap,
                                     mybir.ActivationFunctionType.Copy)
            else:
                nc.vector.tensor_copy(dst_ap, ps_ap)
            di[0] += 1

        # ---- P0b: h0_own = [x|1] @ [Wp;bp] (feature-major, fp32) ----
        for k in range(4):
            for ch in range(2):
                ps = pmm_tile()
                nc.tensor.matmul(ps[:, 0:320], wp_sb[:, 128 * ch:128 * (ch + 1)],
                                 xto_sb[:, 320 * k:320 * (k + 1)],
                                 start=True, stop=True)
                drain_ps(h_own[:, ch, 320 * k:320 * (k + 1)], ps[:, 0:320])

        # ---- xl0_aug for ALL nodes straight from x (contraction dim 8) ----
        for t in range(NP // 128 if STAGE >= 1 else 0):
            ps = pmm_tile()
            nc.tensor.matmul(ps[:, 0:DA], x_sb[:, 128 * t:128 * (t + 1)],
                             wf0_sb[:], start=True, stop=True)
            xsb = drain.tile([128, DA], BF16, tag="xsb")
            drain_ps(xsb[:], ps[:, 0:DA])
            nc.sync.dma_start(xl_d0[128 * t:128 * (t + 1), 0:DA], xsb[:])

        # ---- layers ----
        for l in range(L if STAGE >= 2 else 0):
            # xr = h_own @ Wr -> resident SBUF; relocated to [32, NBLK, D]
            # via SBUF->SBUF DMA so every block's slice sits at partition 0
            # (matmul operand base-partition restriction)
            xr_sb = xrpool.tile([32, NBLK, D], BF16, tag="xr")
            for jo in range(10 if STAGE >= 3 else 0):
                ps = pmm_tile()
                for ch in range(2):
                    nc.tensor.matmul(ps[:, 0:D], h_own[:, ch, 128 * jo:128 * (jo + 1)],
                                     wr_sb[:, l, ch, :],
                                     start=(ch == 0), stop=(ch == 1))
                xrt = drain.tile([128, D], BF16, tag="xrt")
                drain_ps(xrt[:], ps[:, 0:D])
                for g in range(4):
                    nc.sync.dma_start(xr_sb[:, 4 * jo + g, :],
                                      xrt[32 * g:32 * (g + 1), :])

            # edge sweep
            nblk_run = NBLK if STAGE >= 4 else 0
            for b in range(nblk_run):
                xsrc = gath.tile([128, t_blk, DPAD], BF16)
                for g0 in range(0, t_blk, 8):
                    gn = min(8, t_blk - g0)
                    nc.gpsimd.dma_gather(
                        out_ap=xsrc[:, g0:g0 + gn, :],
                        in_ap=xl_d0 if l == 0 else xl_d1,
                        idxs_ap=srcw_sb[:, b * t8 + g0 * 8:b * t8 + (g0 + gn) * 8],
                        num_idxs=gn * PT,
                        num_idxs_reg=gn * PT,
                        elem_size=DPAD,
                    )
                ind_sb = indp.tile([128, t_blk, BLK], BF16)
                nc.sync.dma_start(ind_sb[:], ind_d[:, b * t_blk:(b + 1) * t_blk, :])
                indt_sb = indtp.tile([32, t_blk, PT], BF16)
                nc.sync.dma_start(
                    indt_sb[:], indt_d[:, b * t_blk:(b + 1) * t_blk, :])
                xrb = xr_sb[:, b, :]

                ast = astp.tile([128, t_blk, 8 + D], BF16)
                pb = pblk.tile([32, 8 + D], F32)

                for q0 in range(0, t_blk, QW):
                    w = min(QW, t_blk - q0)
                    pf = pfeat.tile([128, QW, D], F32, tag="pf")
                    for tt in range(q0, q0 + w):
                        k = tt - q0
                        if PRELOAD:
                            nc.scalar.activation(
                                pf[:, k, :], xsrc[:, tt, 0:D],
                                mybir.ActivationFunctionType.Copy)
                            nc.tensor.matmul(pf[:, k, :], indt_sb[:, tt, :],
                                             xrb, start=False, stop=True,
                                             skip_group_check=True)
                        else:
                            nc.tensor.matmul(pf[:, k, :], indt_sb[:, tt, :],
                                             xrb, start=True, stop=False)
                            nc.tensor.matmul(pf[:, k, :], i128_sb[:],
                                             xsrc[:, tt, 0:D],
                                             start=False, stop=True)
                    u = upool.tile([128, QW, D], BF16)
                    nc.scalar.activation(u[:, 0:w, :], pf[:, 0:w, :],
                                         mybir.ActivationFunctionType.Abs)
                    ap_t = appool.tile([128, QW, D], BF16)
                    nc.vector.tensor_mul(ap_t[:, 0:w, :], u[:, 0:w, :],
                                         att_sb[:, l, 0:w, :])
                    e_sb = epool.tile([128, QW, 8], BF16 if EBF16 else F32)
                    if EBF16:
                        with nc.allow_low_precision("e logits tolerate bf16"):
                            nc.vector.tensor_reduce(
                                e_sb[:, 0:w, :],
                                ap_t[:, 0:w, :].rearrange("p a (h c) -> p a h c", h=H),
                                axis=mybir.AxisListType.X, op=mybir.AluOpType.add)
                    else:
                        nc.vector.tensor_reduce(
                            e_sb[:, 0:w, :],
                            ap_t[:, 0:w, :].rearrange("p a (h c) -> p a h c", h=H),
                            axis=mybir.AxisListType.X, op=mybir.AluOpType.add)
                    nc.vector.tensor_add(e_sb[:, 0:w, :], e_sb[:, 0:w, :],
                                         xsrc[:, q0:q0 + w, D:DA])
                    nc.scalar.activation(ast[:, q0:q0 + w, 0:8], e_sb[:, 0:w, :],
                                         mybir.ActivationFunctionType.Exp)
                    nc.vector.tensor_mul(
                        ast[:, q0:q0 + w, 8:].rearrange("p a (h c) -> p a h c", h=H),
                        xsrc[:, q0:q0 + w, 0:D].rearrange("p a (h c) -> p a h c", h=H),
                        ast[:, q0:q0 + w, 0:8].unsqueeze(-1).broadcast_to(
                            [128, w, H, C]))
                    for tt in range(q0, q0 + w):
                        nc.tensor.matmul(pb[:], ind_sb[:, tt, :], ast[:, tt, :],
                                         start=(tt == 0), stop=(tt == t_blk - 1))

                # block epilogue: denom -> 1/(denom+eps); scale; transpose; h +=
                dsb = smallp.tile([32, 8], F32, tag="dsb")
                nc.scalar.activation(dsb[:], pb[:, 0:8],
                                     mybir.ActivationFunctionType.Identity,
                                     bias=eps_sb[:])
                dinv = smallp.tile([32, 8], F32, tag="dinv")
                nc.vector.reciprocal(dinv[:], dsb[:])
                outb = outbp.tile([32, D], F32)
                nc.vector.tensor_mul(
                    outb[:].rearrange("p (h c) -> p h c", h=H),
                    pb[:, 8:].rearrange("p (h c) -> p h c", h=H),
                    dinv[:].unsqueeze(-1).broadcast_to([32, H, C]))
                pt = ptrp.tile([128, 2, 32], F32)
                nc.tensor.transpose(pt[:, 0, :], outb[:, 0:128], i32_sb[:])
                nc.tensor.transpose(pt[:, 1, :], outb[:, 128:256], i32_sb[:])
                for ch in range(2):
                    nc.vector.tensor_add(h_own[:, ch, 32 * b:32 * (b + 1)],
                                         h_own[:, ch, 32 * b:32 * (b + 1)],
                                         pt[:, ch, :])

                # after each group of 8 blocks in layer 0: bconv, project the
                # finished 256 own nodes through Wl1 and AllGather the chunk
                if l == 0 and b % 8 == 7 and STAGE >= 5:
                    j = b // 8
                    for ch in range(2):
                        nc.vector.tensor_scalar_add(
                            h_own[:, ch, CHN * j:CHN * (j + 1)],
                            h_own[:, ch, CHN * j:CHN * (j + 1)],
                            bc_sb[:, 2 * l + ch:2 * l + ch + 1])
                    for half in range(2):
                        n0 = CHN * j + 128 * half
                        ps = pmm_tile()
                        for ch in range(2):
                            nc.tensor.matmul(ps[:, 0:DA],
                                             h_own[:, ch, n0:n0 + 128],
                                             wl1_sb[:, ch, :],
                                             start=(ch == 0), stop=(ch == 1))
                        stg = drain.tile([128, DA], BF16, tag="stg")
                        drain_ps(stg[:], ps[:, 0:DA])
                        nc.sync.dma_start(stage[n0:n0 + 128, 0:DA], stg[:])
                    nc.gpsimd.collective_compute(
                        "AllGather", mybir.AluOpType.bypass,
                        replica_groups=[list(range(NCORES))],
                        ins=[stage[CHN * j:CHN * (j + 1), :]],
                        outs=[xl_d1[NCORES * CHN * j:NCORES * CHN * (j + 1), :]])

            if l == 1 or STAGE < 5:
                # bconv for the whole slice at once
                for ch in range(2):
                    nc.vector.tensor_scalar_add(
                        h_own[:, ch, :], h_own[:, ch, :],
                        bc_sb[:, 2 * l + ch:2 * l + ch + 1])

            if debug:
                nc.sync.dma_start(hdump[l].rearrange("c p n -> p c n"), h_own[:])

        # ---- final: y = h_own @ Wpred ----
        for k in range(3):
            w = 512 if k < 2 else NPC - 1024
            ps = pmm_tile()
            for ch in range(2):
                nc.tensor.matmul(ps[0:1, 0:w], bc_sb[:, 4 + ch:5 + ch],
                                 h_own[:, ch, 512 * k:512 * k + w],
                                 start=(ch == 0), stop=(ch == 1))
            ysb = drain.tile([1, 512], F32, tag="ysb")
            nc.scalar.activation(ysb[0:1, 0:w], ps[0:1, 0:w],
                                 mybir.ActivationFunctionType.Copy)
            nc.sync.dma_start(y_out[0:1, 512 * k:512 * k + w], ysb[0:1, 0:w])

    nc.compile()
    return nc


def _host_inputs(x, Wp, bp, Wl, Wr, att, bconv, Wpred, row_of, perms_old):
    pm = _pad_map()
    xp = np.zeros((NP, F_IN), dtype=np.float32)
    xp[pm] = np.asarray(x, dtype=np.float32)

    # augmented per-layer projections [Wl | Wl@A], A = 0.6*att blockdiag
    wla = []
    for l in range(L):
        A = np.zeros((D, H), dtype=np.float64)
        for h in range(H):
            A[h * C:(h + 1) * C, h] = 0.6 * att[l][h]
        wla.append(np.concatenate([Wl[l], Wl[l] @ A], axis=1))  # [256, 264]

    # layer-0: fold through Wp (x is only 7-dim + ones column for biases)
    wpb = np.concatenate([Wp, bp[None, :]], axis=0)             # [8, 256]
    wf0 = wpb @ wla[0]                                          # [8, 264]

    xaug = np.ones((F_IN + 1, NP), dtype=np.float32)
    xaug[:F_IN, :] = 0.0
    xaug[:F_IN, row_of] = xp.T                                  # chunk-major cols

    wl1_p = np.zeros((128, 2, DA), dtype=np.float32)
    wr_p = np.zeros((128, L, 2, D), dtype=np.float32)
    att_p = np.zeros((128, L, QW, D), dtype=np.float32)
    for ch in range(2):
        wl1_p[:, ch, :] = wla[1][128 * ch:128 * (ch + 1), :]
    for l in range(L):
        for ch in range(2):
            wr_p[:, l, ch, :] = Wr[l][128 * ch:128 * (ch + 1), :]
        a = (0.4 * att[l]).reshape(H * C)
        for q in range(QW):
            att_p[:, l, q, :] = a[None, :]

    bcols = np.zeros((128, 8), dtype=np.float32)
    for ch in range(2):
        for l in range(L):
            bcols[:, 2 * l + ch] = bconv[l][128 * ch:128 * (ch + 1)]
        bcols[:, 4 + ch] = Wpred[128 * ch:128 * (ch + 1), 0]

    shared = {
        "xaug": xaug.astype(NBF),
        "wp": wpb.astype(np.float32),
        "wf0": wf0.astype(NBF),
        "wl1": wl1_p, "wr": wr_p, "attrep": att_p.astype(NBF),
        "bcols": bcols,
        "i32": np.eye(32, dtype=np.float32),
        "i128": np.eye(128, dtype=np.float32).astype(NBF),
        "epsc": np.full((32, 1), 1e-16, dtype=np.float32),
    }
    xtowns = []
    for c in range(NCORES):
        xt = np.ones((F_IN + 1, NPC), dtype=np.float32)
        xt[:F_IN, :] = xp[c * NPC + perms_old[c]].T
        xtowns.append(xt)
    return shared, xtowns


_CACHE = {}


def kernel(x, edge_index, batch, Wp, bp, Wl, Wr, att, bconv, Wpred, bpred,
           debug=False, _timing=None):
    x = np.asarray(x)
    edge_index = np.asarray(edge_index).astype(np.int64)
    batch = np.asarray(batch).astype(np.int64)

    t_blk, perms_old, perms_new, row_of, cores = _prep_edges(edge_index)
    shared, xtowns = _host_inputs(np.asarray(x), np.asarray(Wp), np.asarray(bp),
                                  np.asarray(Wl), np.asarray(Wr), np.asarray(att),
                                  np.asarray(bconv), np.asarray(Wpred),
                                  row_of, perms_old)

    key = (t_blk, bool(debug), os.environ.get("GAT_STAGE", "99"),
           os.environ.get("GAT_PRELOAD", "1"), os.environ.get("GAT_EBF16", "1"))
    if key not in _CACHE:
        _CACHE[key] = _build(t_blk, debug=debug)
    nc = _CACHE[key]

    in_maps = []
    for c in range(NCORES):
        m = dict(shared)
        m["xtown"] = xtowns[c]
        m["srcw"] = cores[c]["srcw"]
        m["ind"] = cores[c]["ind_pm"]
        m["indt"] = cores[c]["indt_pm"]
        in_maps.append(m)

    kw = {k: v for k, v in (_timing or {}).items() if k != "result"}
    res = run_bass_kernel_spmd(nc, in_maps, list(range(NCORES)), **kw)
    if _timing is not None:
        _timing["result"] = res

    pm = _pad_map()
    ycat = np.stack([res.results[c]["y_out"][0] for c in range(NCORES)])
    p = pm
    c_of = p // NPC
    r_new = np.concatenate([perms_new[c][None] for c in range(NCORES)])  # [8, NPC]
    y_real = ycat[c_of, r_new[c_of, p % NPC]]
    sums = np.bincount(batch, weights=y_real.astype(np.float64), minlength=G)
    cnt = np.bincount(batch, minlength=G).astype(np.float64)
    out = sums / np.maximum(cnt, 1.0) + float(np.asarray(bpred).reshape(-1)[0])
    if debug:
        return out.astype(np.float32)[:, None], res
    return out.astype(np.float32)[:, None]


# revision 11
# speedup vs baseline: 1.7817x; 1.3665x over previous
"""GATv2 (2-layer, 8-head) message-passing kernel for 8 Trainium2 NeuronCores.

Sharding: nodes (and incoming edges) are partitioned across 8 cores by
destination; weights replicated.  Per core, dst nodes are LPT-bin-packed into
40 blocks of 32 nodes (balancing edge counts so every block needs the same
t_blk=8 tiles of 128 edges).  Layer-0 source features xl0 are computed
replicated directly from x via the host-folded weight [Wp;bp]@[Wl|Wl@A]
(contraction dim 8), written to DRAM, and gathered per edge with SWDGE.
Layer-1 xl is NOT recomputed replicated: each core projects its own 1280
updated nodes and a chunked AllGather (5 chunks of 8 blocks, issued as the
edge-0 sweep retires blocks) distributes them, overlapping the collective
with compute.

Edge math per 128-edge tile: ACT preloads the gathered xl into PSUM, one PE
matmul accumulates the indicator-expanded xr on top (feat = xl[src]+xr[dst]);
ACT abs, DVE mul by 0.4*att + per-head reduce (lrelu folded as 0.6x+0.4|x|
with the 0.6-linear source term riding in 8 extra gathered columns; the dst
linear term cancels in the softmax), ACT exp, DVE alpha*xl, and a PE
indicator-matmul scatters [a | msg] into per-block PSUM accumulators.

bf16 everywhere on the edge path; accumulations (PSUM, h residual) fp32.
Self-contained: takes full (unsharded) inputs, returns the full output.
"""

import os
import numpy as np
from contextlib import ExitStack

import ml_dtypes
import concourse.bass as bass
import concourse.tile as tile
from concourse import bacc, mybir
from concourse.bass_utils import run_bass_kernel_spmd

# Problem constants (from the nn_GATv2Model spec)
N = 10000
E = 320000
F_IN = 7
D = 256
H = 8
C = 32
L = 2
G = 16
SLOPE = 0.2

NCORES = 8
NPC_REAL = 1250          # real nodes per core
NPC = 1280               # padded nodes per core (multiple of 256)
NP = NCORES * NPC        # padded total nodes
BLK = 32                 # dst nodes per block (indicator-matmul segment group)
NBLK = NPC // BLK        # blocks per core
PT = 128                 # edges per tile (partition dim)
CH = 5                   # AllGather chunks (NPC/256)
CHN = 256                # nodes per chunk
QW = 4                   # tiles per PSUM quad-group
DA = D + 8               # payload cols: 256 feat + 8 source-linear
DPAD = 384               # xl_d row pitch in bf16 elems (768B, 256B multiple)

F32 = mybir.dt.float32
BF16 = mybir.dt.bfloat16
I16 = mybir.dt.int16
NBF = ml_dtypes.bfloat16


def _pad_map():
    n = np.arange(N)
    return (n // NPC_REAL) * NPC + (n % NPC_REAL)


def _prep_edges(edge_index: np.ndarray):
    """LPT-pack dst nodes into blocks, sort/shard/pad edges.

    Returns (t_blk, perms, cores) where perms[c] is old_of_new (new local
    id -> old padded-local id) and cores[c] has srcw/ind_pm/indt_pm."""
    pm = _pad_map()
    src_p = pm[edge_index[0]]
    dst_p = pm[edge_index[1]]
    owner = dst_p // NPC

    # ---- per-core LPT bin-packing of dst nodes into NBLK bins of 32 ----
    perms_new = []      # new_of_old per core
    perms_old = []      # old_of_new per core
    core_sel = []
    t_blk = 1
    for c in range(NCORES):
        sel = owner == c
        core_sel.append(sel)
        dl = dst_p[sel] - c * NPC
        deg = np.bincount(dl, minlength=NPC)
        order = np.argsort(-deg, kind="stable")
        binload = np.zeros(NBLK, np.int64)
        bincnt = np.zeros(NBLK, np.int64)
        assign = np.zeros(NPC, np.int64)
        for node in order:
            open_b = np.nonzero(bincnt < BLK)[0]
            b = open_b[np.argmin(binload[open_b])]
            assign[node] = b
            binload[b] += deg[node]
            bincnt[b] += 1
        new_of_old = np.zeros(NPC, np.int64)
        pos = np.zeros(NBLK, np.int64)
        for old in range(NPC):
            b = assign[old]
            new_of_old[old] = b * BLK + pos[b]
            pos[b] += 1
        perms_new.append(new_of_old)
        perms_old.append(np.argsort(new_of_old))
        t_blk = max(t_blk, int(np.ceil(binload.max() / PT)))

    # global chunk-major row map: padded id -> xl_d row
    row_of = np.zeros(NP, np.int64)
    for c in range(NCORES):
        r = perms_new[c]                      # old local -> new local
        j = r // CHN
        row_of[c * NPC:(c + 1) * NPC] = (j * NCORES + c) * CHN + (r % CHN)

    t8 = t_blk * 8
    ne_slots = NBLK * t_blk * PT
    cores = []
    for c in range(NCORES):
        sel = core_sel[c]
        es = row_of[src_p[sel]]                       # global xl_d rows
        ed = perms_new[c][dst_p[sel] - c * NPC]       # new local dst
        order = np.argsort(ed, kind="stable")
        es, ed = es[order], ed[order]
        blk = ed // BLK
        cnt = np.bincount(blk, minlength=NBLK)
        starts = np.zeros(NBLK, dtype=np.int64)
        starts[1:] = np.cumsum(cnt)[:-1]
        within = np.arange(len(es)) - starts[blk]
        slot = blk * (t_blk * PT) + within

        src_slots = np.zeros(ne_slots, dtype=np.int64)
        dl32 = np.full(ne_slots, -1, dtype=np.int64)
        src_slots[slot] = es
        dl32[slot] = ed - blk * BLK

        # wrapped gather indices [128, ne_slots//16]
        w16 = np.concatenate(
            [src_slots[b * t_blk * PT:(b + 1) * t_blk * PT]
             .reshape(t_blk * 8, 16).T
             for b in range(NBLK)], axis=1).astype(np.int16)
        srcw = np.tile(w16, (8, 1))

        # indicator [NT, 128, 32] -> partition-major [128, NT, 32] bf16
        nt = NBLK * t_blk
        ind = np.zeros((ne_slots, BLK), dtype=np.float32)
        valid = dl32 >= 0
        ind[np.nonzero(valid)[0], dl32[valid]] = 1.0
        ind = ind.reshape(nt, PT, BLK)
        ind_pm = np.ascontiguousarray(ind.transpose(1, 0, 2)).astype(NBF)
        indt_pm = np.ascontiguousarray(
            ind.transpose(0, 2, 1).transpose(1, 0, 2)).astype(NBF)
        cores.append({"srcw": srcw, "ind_pm": ind_pm, "indt_pm": indt_pm})
    return t_blk, perms_old, perms_new, row_of, cores


def _build(t_blk: int, debug: bool = False):
    """Build + compile the SPMD Bass program (identical on all cores)."""
    STAGE = int(os.environ.get("GAT_STAGE", "99"))
    PRELOAD = os.environ.get("GAT_PRELOAD", "1") == "1"
    EBF16 = os.environ.get("GAT_EBF16", "1") == "1"
    nt = NBLK * t_blk
    t8 = t_blk * 8

    nc = bacc.Bacc("TRN2", target_bir_lowering=False, debug=False,
                   num_devices=NCORES, num_swdge_queues=4)

    # ---- external inputs ----
    xaug = nc.dram_tensor("xaug", [F_IN + 1, NP], BF16, kind="ExternalInput").ap()
    xtown = nc.dram_tensor("xtown", [F_IN + 1, NPC], F32, kind="ExternalInput").ap()
    wp = nc.dram_tensor("wp", [F_IN + 1, D], F32, kind="ExternalInput").ap()
    wf0 = nc.dram_tensor("wf0", [F_IN + 1, DA], BF16, kind="ExternalInput").ap()
    wl1 = nc.dram_tensor("wl1", [128, 2, DA], F32, kind="ExternalInput").ap()
    wr = nc.dram_tensor("wr", [128, L, 2, D], F32, kind="ExternalInput").ap()
    attrep = nc.dram_tensor("attrep", [128, L, QW, D], BF16, kind="ExternalInput").ap()
    bcols = nc.dram_tensor("bcols", [128, 8], F32, kind="ExternalInput").ap()
    i32 = nc.dram_tensor("i32", [32, 32], F32, kind="ExternalInput").ap()
    epsc = nc.dram_tensor("epsc", [32, 1], F32, kind="ExternalInput").ap()
    i128 = nc.dram_tensor("i128", [128, 128], BF16, kind="ExternalInput").ap()
    srcw_d = nc.dram_tensor("srcw", [128, NBLK * t8], I16, kind="ExternalInput").ap()
    ind_d = nc.dram_tensor("ind", [128, nt, BLK], BF16, kind="ExternalInput").ap()
    indt_d = nc.dram_tensor("indt", [32, nt, PT], BF16, kind="ExternalInput").ap()

    # ---- internal DRAM ----
    xl_d0 = nc.dram_tensor("xl_d0", [NP, DPAD], BF16).ap()
    stage = nc.dram_tensor("stage", [NPC, DPAD], BF16).ap()
    xl_d1 = nc.dram_tensor("xl_d1", [NP, DPAD], BF16, addr_space="Shared").ap()

    # ---- outputs ----
    y_out = nc.dram_tensor("y_out", [1, NPC], F32, kind="ExternalOutput").ap()
    if debug:
        hdump = nc.dram_tensor("hdump", [L, 2, 128, NPC], F32, kind="ExternalOutput").ap()

    with tile.TileContext(nc) as tc, ExitStack() as ctx:
        cpool = ctx.enter_context(tc.tile_pool(name="consts", bufs=1))
        drain = ctx.enter_context(tc.tile_pool(name="drain", bufs=4))
        gath = ctx.enter_context(tc.tile_pool(name="gath", bufs=4))
        indp = ctx.enter_context(tc.tile_pool(name="indp", bufs=4))
        indtp = ctx.enter_context(tc.tile_pool(name="indtp", bufs=4))
        xrpool = ctx.enter_context(tc.tile_pool(name="xrpool", bufs=1))
        upool = ctx.enter_context(tc.tile_pool(name="upool", bufs=4))
        appool = ctx.enter_context(tc.tile_pool(name="appool", bufs=4))
        epool = ctx.enter_context(tc.tile_pool(name="epool", bufs=4))
        astp = ctx.enter_context(tc.tile_pool(name="astp", bufs=4))
        smallp = ctx.enter_context(tc.tile_pool(name="smallp", bufs=3))
        outbp = ctx.enter_context(tc.tile_pool(name="outbp", bufs=3))

        # PSUM (8 banks): pfeat 2x2=4, pblk 3, ptr 1
        pfeat = ctx.enter_context(tc.tile_pool(name="pfeat", bufs=2, space="PSUM"))
        pblk = ctx.enter_context(tc.tile_pool(name="pblk", bufs=3, space="PSUM"))
        ptrp = ctx.enter_context(tc.tile_pool(name="ptrp", bufs=1, space="PSUM"))

        def pmm_tile():
            t = pfeat.tile([128, QW, D], F32, tag="pf")
            return t[:].rearrange("p a b -> p (a b)")

        # ---- resident SBUF constants ----
        x_sb = cpool.tile([F_IN + 1, NP], BF16)
        nc.sync.dma_start(x_sb[:], xaug)
        xto_sb = cpool.tile([F_IN + 1, NPC], F32)
        nc.sync.dma_start(xto_sb[:], xtown)
        wp_sb = cpool.tile([F_IN + 1, D], F32)
        nc.sync.dma_start(wp_sb[:], wp)
        wf0_sb = cpool.tile([F_IN + 1, DA], BF16)
        nc.sync.dma_start(wf0_sb[:], wf0)
        wl1_sb = cpool.tile([128, 2, DA], F32)
        nc.sync.dma_start(wl1_sb[:], wl1)
        wr_sb = cpool.tile([128, L, 2, D], F32)
        nc.sync.dma_start(wr_sb[:], wr)
        att_sb = cpool.tile([128, L, QW, D], BF16)
        nc.sync.dma_start(att_sb[:], attrep)
        bc_sb = cpool.tile([128, 8], F32)
        nc.sync.dma_start(bc_sb[:], bcols)
        i32_sb = cpool.tile([32, 32], F32)
        nc.sync.dma_start(i32_sb[:], i32)
        eps_sb = cpool.tile([32, 1], F32)
        nc.sync.dma_start(eps_sb[:], epsc)
        i128_sb = cpool.tile([128, 128], BF16)
        nc.sync.dma_start(i128_sb[:], i128)
        srcw_sb = cpool.tile([128, NBLK * t8], I16)
        nc.sync.dma_start(srcw_sb[:], srcw_d)

        h_own = cpool.tile([128, 2, NPC], F32)     # h own slice, resident

        di = [0]

        def drain_ps(dst_ap, ps_ap):
            """PSUM -> SBUF copy alternating ACT/DVE to balance load."""
            if di[0] % 2 == 0:
                nc.scalar.activation(dst_ap, ps_ap,
                                     mybir.ActivationFunctionType.Copy)
            else:
                nc.vector.tensor_copy(dst_ap, ps_ap)
            di[0] += 1

        # ---- P0b: h0_own = [x|1] @ [Wp;bp] (feature-major, fp32) ----
        for k in range(4):
            for ch in range(2):
                ps = pmm_tile()
                nc.tensor.matmul(ps[:, 0:320], wp_sb[:, 128 * ch:128 * (ch + 1)],
                                 xto_sb[:, 320 * k:320 * (k + 1)],
                                 start=True, stop=True)
                drain_ps(h_own[:, ch, 320 * k:320 * (k + 1)], ps[:, 0:320])

        # ---- xl0_aug for ALL nodes straight from x (contraction dim 8) ----
        for t in range(NP // 128 if STAGE >= 1 else 0):
            ps = pmm_tile()
            nc.tensor.matmul(ps[:, 0:DA], x_sb[:, 128 * t:128 * (t + 1)],
                             wf0_sb[:], start=True, stop=True)
            xsb = drain.tile([128, DA], BF16, tag="xsb")
            drain_ps(xsb[:], ps[:, 0:DA])
            nc.sync.dma_start(xl_d0[128 * t:128 * (t + 1), 0:DA], xsb[:])

        # ---- layers ----
        for l in range(L if STAGE >= 2 else 0):
            # xr = h_own @ Wr -> resident SBUF; relocated to [32, NBLK, D]
            # via SBUF->SBUF DMA so every block's slice sits at partition 0
            # (matmul operand base-partition restriction)
            xr_sb = xrpool.tile([32, NBLK, D], BF16, tag="xr")
            for jo in range(10 if STAGE >= 3 else 0):
                ps = pmm_tile()
                for ch in range(2):
                    nc.tensor.matmul(ps[:, 0:D], h_own[:, ch, 128 * jo:128 * (jo + 1)],
                                     wr_sb[:, l, ch, :],
                                     start=(ch == 0), stop=(ch == 1))
                xrt = drain.tile([128, D], BF16, tag="xrt")
                drain_ps(xrt[:], ps[:, 0:D])
                for g in range(4):
                    nc.sync.dma_start(xr_sb[:, 4 * jo + g, :],
                                      xrt[32 * g:32 * (g + 1), :])

            # edge sweep
            nblk_run = NBLK if STAGE >= 4 else 0
            nhalf = (t_blk + QW - 1) // QW
            for b in range(nblk_run):
                # gather each QW-tile half into its own tile on its own SWDGE
                # queue: halves the time-to-first-data and runs the random-row
                # HBM transfers on parallel rings
                xs = []
                for half in range(nhalf):
                    g0 = half * QW
                    gn = min(QW, t_blk - g0)
                    xh = gath.tile([128, QW, DPAD], BF16, tag=f"g{half}")
                    nc.gpsimd.dma_gather(
                        out_ap=xh[:, 0:gn, :],
                        in_ap=xl_d0 if l == 0 else xl_d1,
                        idxs_ap=srcw_sb[:, b * t8 + g0 * 8:b * t8 + (g0 + gn) * 8],
                        num_idxs=gn * PT,
                        num_idxs_reg=gn * PT,
                        elem_size=DPAD,
                        queue_num=(nhalf * b + half) % 4,
                    )
                    xs.append(xh)
                ind_sb = indp.tile([128, t_blk, BLK], BF16)
                nc.sync.dma_start(ind_sb[:], ind_d[:, b * t_blk:(b + 1) * t_blk, :])
                indt_sb = indtp.tile([32, t_blk, PT], BF16)
                nc.sync.dma_start(
                    indt_sb[:], indt_d[:, b * t_blk:(b + 1) * t_blk, :])
                xrb = xr_sb[:, b, :]

                ast = astp.tile([128, t_blk, 8 + D], BF16)
                pb = pblk.tile([32, 8 + D], F32)

                for q0 in range(0, t_blk, QW):
                    w = min(QW, t_blk - q0)
                    xsrc = xs[q0 // QW]
                    pf = pfeat.tile([128, QW, D], F32, tag="pf")
                    for k in range(w):
                        tt = q0 + k
                        if PRELOAD:
                            nc.scalar.activation(
                                pf[:, k, :], xsrc[:, k, 0:D],
                                mybir.ActivationFunctionType.Copy)
                            nc.tensor.matmul(pf[:, k, :], indt_sb[:, tt, :],
                                             xrb, start=False, stop=True,
                                             skip_group_check=True)
                        else:
                            nc.tensor.matmul(pf[:, k, :], indt_sb[:, tt, :],
                                             xrb, start=True, stop=False)
                            nc.tensor.matmul(pf[:, k, :], i128_sb[:],
                                             xsrc[:, k, 0:D],
                                             start=False, stop=True)
                    u = upool.tile([128, QW, D], BF16)
                    nc.scalar.activation(u[:, 0:w, :], pf[:, 0:w, :],
                                         mybir.ActivationFunctionType.Abs)
                    ap_t = appool.tile([128, QW, D], BF16)
                    nc.vector.tensor_mul(ap_t[:, 0:w, :], u[:, 0:w, :],
                                         att_sb[:, l, 0:w, :])
                    e_sb = epool.tile([128, QW, 8], BF16 if EBF16 else F32)
                    if EBF16:
                        with nc.allow_low_precision("e logits tolerate bf16"):
                            nc.vector.tensor_reduce(
                                e_sb[:, 0:w, :],
                                ap_t[:, 0:w, :].rearrange("p a (h c) -> p a h c", h=H),
                                axis=mybir.AxisListType.X, op=mybir.AluOpType.add)
                    else:
                        nc.vector.tensor_reduce(
                            e_sb[:, 0:w, :],
                            ap_t[:, 0:w, :].rearrange("p a (h c) -> p a h c", h=H),
                            axis=mybir.AxisListType.X, op=mybir.AluOpType.add)
                    nc.vector.tensor_add(e_sb[:, 0:w, :], e_sb[:, 0:w, :],
                                         xsrc[:, 0:w, D:DA])
                    nc.scalar.activation(ast[:, q0:q0 + w, 0:8], e_sb[:, 0:w, :],
                                         mybir.ActivationFunctionType.Exp)
                    nc.vector.tensor_mul(
                        ast[:, q0:q0 + w, 8:].rearrange("p a (h c) -> p a h c", h=H),
                        xsrc[:, 0:w, 0:D].rearrange("p a (h c) -> p a h c", h=H),
                        ast[:, q0:q0 + w, 0:8].unsqueeze(-1).broadcast_to(
                            [128, w, H, C]))
                    for tt in range(q0, q0 + w):
                        nc.tensor.matmul(pb[:], ind_sb[:, tt, :], ast[:, tt, :],
                                         start=(tt == 0), stop=(tt == t_blk - 1))

                # block epilogue: denom -> 1/(denom+eps); scale; transpose; h +=
                dsb = smallp.tile([32, 8], F32, tag="dsb")
                nc.scalar.activation(dsb[:], pb[:, 0:8],
                                     mybir.ActivationFunctionType.Identity,
                                     bias=eps_sb[:])
                dinv = smallp.tile([32, 8], F32, tag="dinv")
                nc.vector.reciprocal(dinv[:], dsb[:])
                outb = outbp.tile([32, D], F32)
                nc.vector.tensor_mul(
                    outb[:].rearrange("p (h c) -> p h c", h=H),
                    pb[:, 8:].rearrange("p (h c) -> p h c", h=H),
                    dinv[:].unsqueeze(-1).broadcast_to([32, H, C]))
                pt = ptrp.tile([128, 2, 32], F32)
                nc.tensor.transpose(pt[:, 0, :], outb[:, 0:128], i32_sb[:])
                nc.tensor.transpose(pt[:, 1, :], outb[:, 128:256], i32_sb[:])
                for ch in range(2):
                    nc.vector.tensor_add(h_own[:, ch, 32 * b:32 * (b + 1)],
                                         h_own[:, ch, 32 * b:32 * (b + 1)],
                                         pt[:, ch, :])

                # after each group of 8 blocks in layer 0: bconv, project the
                # finished 256 own nodes through Wl1 and AllGather the chunk
                if l == 0 and b % 8 == 7 and STAGE >= 5:
                    j = b // 8
                    for ch in range(2):
                        nc.vector.tensor_scalar_add(
                            h_own[:, ch, CHN * j:CHN * (j + 1)],
                            h_own[:, ch, CHN * j:CHN * (j + 1)],
                            bc_sb[:, 2 * l + ch:2 * l + ch + 1])
                    for half in range(2):
                        n0 = CHN * j + 128 * half
                        ps = pmm_tile()
                        for ch in range(2):
                            nc.tensor.matmul(ps[:, 0:DA],
                                             h_own[:, ch, n0:n0 + 128],
                                             wl1_sb[:, ch, :],
                                             start=(ch == 0), stop=(ch == 1))
                        stg = drain.tile([128, DA], BF16, tag="stg")
                        drain_ps(stg[:], ps[:, 0:DA])
                        nc.sync.dma_start(stage[n0:n0 + 128, 0:DA], stg[:])
                    nc.gpsimd.collective_compute(
                        "AllGather", mybir.AluOpType.bypass,
                        replica_groups=[list(range(NCORES))],
                        ins=[stage[CHN * j:CHN * (j + 1), :]],
                        outs=[xl_d1[NCORES * CHN * j:NCORES * CHN * (j + 1), :]])

            if l == 1 or STAGE < 5:
                # bconv for the whole slice at once
                for ch in range(2):
                    nc.vector.tensor_scalar_add(
                        h_own[:, ch, :], h_own[:, ch, :],
                        bc_sb[:, 2 * l + ch:2 * l + ch + 1])

            if debug:
                nc.sync.dma_start(hdump[l].rearrange("c p n -> p c n"), h_own[:])

        # ---- final: y = h_own @ Wpred ----
        for k in range(3):
            w = 512 if k < 2 else NPC - 1024
            ps = pmm_tile()
            for ch in range(2):
                nc.tensor.matmul(ps[0:1, 0:w], bc_sb[:, 4 + ch:5 + ch],
                                 h_own[:, ch, 512 * k:512 * k + w],
                                 start=(ch == 0), stop=(ch == 1))
            ysb = drain.tile([1, 512], F32, tag="ysb")
            nc.scalar.activation(ysb[0:1, 0:w], ps[0:1, 0:w],
                                 mybir.ActivationFunctionType.Copy)
            nc.sync.dma_start(y_out[0:1, 512 * k:512 * k + w], ysb[0:1, 0:w])

    nc.compile()
    return nc


def _host_inputs(x, Wp, bp, Wl, Wr, att, bconv, Wpred, row_of, perms_old):
    pm = _pad_map()
    xp = np.zeros((NP, F_IN), dtype=np.float32)
    xp[pm] = np.asarray(x, dtype=np.float32)

    # augmented per-layer projections [Wl | Wl@A], A = 0.6*att blockdiag
    wla = []
    for l in range(L):
        A = np.zeros((D, H), dtype=np.float64)
        for h in range(H):
            A[h * C:(h + 1) * C, h] = 0.6 * att[l][h]
        wla.append(np.concatenate([Wl[l], Wl[l] @ A], axis=1))  # [256, 264]

    # layer-0: fold through Wp (x is only 7-dim + ones column for biases)
    wpb = np.concatenate([Wp, bp[None, :]], axis=0)             # [8, 256]
    wf0 = wpb @ wla[0]                                          # [8, 264]

    xaug = np.ones((F_IN + 1, NP), dtype=np.float32)
    xaug[:F_IN, :] = 0.0
    xaug[:F_IN, row_of] = xp.T                                  # chunk-major cols

    wl1_p = np.zeros((128, 2, DA), dtype=np.float32)
    wr_p = np.zeros((128, L, 2, D), dtype=np.float32)
    att_p = np.zeros((128, L, QW, D), dtype=np.float32)
    for ch in range(2):
        wl1_p[:, ch, :] = wla[1][128 * ch:128 * (ch + 1), :]
    for l in range(L):
        for ch in range(2):
            wr_p[:, l, ch, :] = Wr[l][128 * ch:128 * (ch + 1), :]
        a = (0.4 * att[l]).reshape(H * C)
        for q in range(QW):
            att_p[:, l, q, :] = a[None, :]

    bcols = np.zeros((128, 8), dtype=np.float32)
    for ch in range(2):
        for l in range(L):
            bcols[:, 2 * l + ch] = bconv[l][128 * ch:128 * (ch + 1)]
        bcols[:, 4 + ch] = Wpred[128 * ch:128 * (ch + 1), 0]

    shared = {
        "xaug": xaug.astype(NBF),
        "wp": wpb.astype(np.float32),
        "wf0": wf0.astype(NBF),
        "wl1": wl1_p, "wr": wr_p, "attrep": att_p.astype(NBF),
        "bcols": bcols,
        "i32": np.eye(32, dtype=np.float32),
        "i128": np.eye(128, dtype=np.float32).astype(NBF),
        "epsc": np.full((32, 1), 1e-16, dtype=np.float32),
    }
    xtowns = []
    for c in range(NCORES):
        xt = np.ones((F_IN + 1, NPC), dtype=np.float32)
        xt[:F_IN, :] = xp[c * NPC + perms_old[c]].T
        xtowns.append(xt)
    return shared, xtowns


_CACHE = {}


def kernel(x, edge_index, batch, Wp, bp, Wl, Wr, att, bconv, Wpred, bpred,
           debug=False, _timing=None):
    x = np.asarray(x)
    edge_index = np.asarray(edge_index).astype(np.int64)
    batch = np.asarray(batch).astype(np.int64)

    t_blk, perms_old, perms_new, row_of, cores = _prep_edges(edge_index)
    shared, xtowns = _host_inputs(np.asarray(x), np.asarray(Wp), np.asarray(bp),
                                  np.asarray(Wl), np.asarray(Wr), np.asarray(att),
                                  np.asarray(bconv), np.asarray(Wpred),
                                  row_of, perms_old)

    key = (t_blk, bool(debug), os.environ.get("GAT_STAGE", "99"),
           os.environ.get("GAT_PRELOAD", "1"), os.environ.get("GAT_EBF16", "1"))
    if key not in _CACHE:
        _CACHE[key] = _build(t_blk, debug=debug)
    nc = _CACHE[key]

    in_maps = []
    for c in range(NCORES):
        m = dict(shared)
        m["xtown"] = xtowns[c]
        m["srcw"] = cores[c]["srcw"]
        m["ind"] = cores[c]["ind_pm"]
        m["indt"] = cores[c]["indt_pm"]
        in_maps.append(m)

    kw = {k: v for k, v in (_timing or {}).items() if k != "result"}
    res = run_bass_kernel_spmd(nc, in_maps, list(range(NCORES)), **kw)
    if _timing is not None:
        _timing["result"] = res

    pm = _pad_map()
    ycat = np.stack([res.results[c]["y_out"][0] for c in range(NCORES)])
    p = pm
    c_of = p // NPC
    r_new = np.concatenate([perms_new[c][None] for c in range(NCORES)])  # [8, NPC]
    y_real = ycat[c_of, r_new[c_of, p % NPC]]
    sums = np.bincount(batch, weights=y_real.astype(np.float64), minlength=G)
    cnt = np.bincount(batch, minlength=G).astype(np.float64)
    out = sums / np.maximum(cnt, 1.0) + float(np.asarray(bpred).reshape(-1)[0])
    if debug:
        return out.astype(np.float32)[:, None], res
    return out.astype(np.float32)[:, None]
